# revision 1
# baseline (speedup 1.0000x reference)
import os

# fp32 matmuls on device: the 8-tick recurrence + layernorms amplify bf16
# matmul error, so disable the neuron compiler's auto-downcast.
os.environ.setdefault(
    "NEURON_CC_FLAGS",
    "--auto-cast=none --model-type=transformer --optlevel=1",
)

import math

import jax
import jax.numpy as jnp
import numpy as np

# ---- model constants (hardcoded; must match the generating config) ----
D = 1024
H = 16
HD = D // H
K = 8
NS = 512
G = D // 4
N_IN = D // 5            # 204
N_AT = int(D * 0.3)      # 307
N_OUT = int(D * 0.3)     # 307
N_MO = D - N_IN - N_AT - N_OUT  # 206
M_IN = 4
M_AT = 128
M_OUT = 16
M_MO = 4
NLM_H = 16
B = 2
T = 512
T_IN = 2
T_MO = 2
N_CORES = 8
TCH = T // (N_CORES // B)  # 128 rows per core
SCALE = 1.0 / math.sqrt(HD)


def _glu(z):
    a, b = jnp.split(z, 2, axis=-1)
    return a * jax.nn.sigmoid(b)


def _rms(x, eps=1e-6):
    return x * jax.lax.rsqrt(jnp.mean(x * x, axis=-1, keepdims=True) + eps)


def _ln(x, g, b, eps=1e-5):
    mu = jnp.mean(x, axis=-1, keepdims=True)
    var = jnp.mean((x - mu) ** 2, axis=-1, keepdims=True)
    return (x - mu) * jax.lax.rsqrt(var + eps) * g + b


def _nlm(tr, w, b):
    # tr: (R, n, m); w: (m, o, n); b: (n, o) -> GLU-halved (R, n, o/2)
    return _glu(jnp.einsum('bnm,mon->bno', tr, w) + b)


def _shard_forward(x_b, rows, Wq, Wk, Wv, Wg, W_in, ln_in_g, ln_in_b,
                   W_at, ln_at_g, ln_at_b, W_out, ln_out_g, ln_out_b,
                   W_mo, ln_mo_g, ln_mo_b, tick_embed, decay_out, decay_act,
                   Wc, in_w1, in_b1, in_s0, in_tr0, at_w1, at_b1, at_w2,
                   at_b2, at_s0, at_tr0, out_w1, out_b1, out_w2, out_b2,
                   out_s0, out_tr0, mo_w1, mo_b1, mo_s0, mo_tr0,
                   idx_out_l, idx_out_r, idx_act_l, idx_act_r):
    # One core's shard: `rows` are TCH absolute token positions of one batch
    # element; x_b is that batch element's full (T, D) input so keys/values
    # cover the whole causal prefix without cross-core traffic.
    R = TCH
    kk = _rms((x_b @ Wk).reshape(T, H, HD))
    vv = (x_b @ Wv).reshape(T, H, HD)
    mask = rows[:, None] >= jnp.arange(T)[None, :]  # (R, T) causal

    state = jnp.broadcast_to(jnp.concatenate([in_s0, at_s0, out_s0, mo_s0]), (R, D))
    tr_in = jnp.broadcast_to(in_tr0, (R, N_IN, M_IN))
    tr_at = jnp.broadcast_to(at_tr0, (R, N_AT, M_AT))
    tr_out = jnp.broadcast_to(out_tr0, (R, N_OUT, M_OUT))
    tr_mo = jnp.broadcast_to(mo_tr0, (R, N_MO, M_MO))

    alpha_out = state[:, idx_out_l] * state[:, idx_out_r]
    beta_out = jnp.ones_like(alpha_out)
    alpha_act = state[:, idx_act_l] * state[:, idx_act_r]
    beta_act = jnp.ones_like(alpha_act)
    d_out = jnp.exp(-jnp.clip(decay_out, 0.0, 15.0))
    d_act = jnp.exp(-jnp.clip(decay_act, 0.0, 15.0))

    s_in = state[:, :N_IN]
    s_at = state[:, N_IN:N_IN + N_AT]
    s_out = state[:, N_IN + N_AT:N_IN + N_AT + N_OUT]
    s_mo = state[:, N_IN + N_AT + N_OUT:]

    for t in range(K):
        sync_act = alpha_act * jax.lax.rsqrt(beta_act)
        q = _rms((sync_act @ Wq).reshape(R, H, HD))
        sc = jnp.einsum('qhd,khd->hqk', q, kk) * SCALE
        sc = jnp.where(mask[None, :, :], sc, -1e30)
        attn = jnp.einsum('hqk,khd->qhd', jax.nn.softmax(sc, axis=-1), vv)
        attn = attn.reshape(R, D) + tick_embed[t]
        gfeat = _glu(state @ Wg)

        if t < T_IN:
            pre = _ln(_glu(jnp.concatenate([attn, s_mo, gfeat], -1) @ W_in), ln_in_g, ln_in_b)
            tr_in = jnp.concatenate([tr_in[:, :, 1:], pre[:, :, None]], -1)
            s_in = _nlm(tr_in, in_w1, in_b1)[..., 0]

        pre = _ln(_glu(jnp.concatenate([s_in, attn], -1) @ W_at), ln_at_g, ln_at_b)
        tr_at = jnp.concatenate([tr_at[:, :, 1:], pre[:, :, None]], -1)
        s_at = _nlm(_nlm(tr_at, at_w1, at_b1), at_w2, at_b2)[..., 0]

        pre = _ln(_glu(jnp.concatenate([s_at, attn, gfeat], -1) @ W_out), ln_out_g, ln_out_b)
        tr_out = jnp.concatenate([tr_out[:, :, 1:], pre[:, :, None]], -1)
        s_out = _nlm(_nlm(tr_out, out_w1, out_b1), out_w2, out_b2)[..., 0]

        if t < T_MO:
            pre = _ln(_glu(jnp.concatenate([s_out, gfeat], -1) @ W_mo), ln_mo_g, ln_mo_b)
            tr_mo = jnp.concatenate([tr_mo[:, :, 1:], pre[:, :, None]], -1)
            s_mo = _nlm(tr_mo, mo_w1, mo_b1)[..., 0]

        state = jnp.concatenate([s_in, s_at, s_out, s_mo], -1)
        alpha_out = d_out * alpha_out + state[:, idx_out_l] * state[:, idx_out_r]
        beta_out = d_out * beta_out + 1.0
        alpha_act = d_act * alpha_act + state[:, idx_act_l] * state[:, idx_act_r]
        beta_act = d_act * beta_act + 1.0

    sync_out = alpha_out * jax.lax.rsqrt(beta_out)
    return sync_out @ Wc  # (R, D)


_WEIGHT_ORDER = (
    'Wq', 'Wk', 'Wv', 'Wg', 'W_in', 'ln_in_g', 'ln_in_b', 'W_at', 'ln_at_g',
    'ln_at_b', 'W_out', 'ln_out_g', 'ln_out_b', 'W_mo', 'ln_mo_g', 'ln_mo_b',
    'tick_embed', 'decay_out', 'decay_act', 'Wc', 'in_w1', 'in_b1', 'in_s0',
    'in_tr0', 'at_w1', 'at_b1', 'at_w2', 'at_b2', 'at_s0', 'at_tr0',
    'out_w1', 'out_b1', 'out_w2', 'out_b2', 'out_s0', 'out_tr0', 'mo_w1',
    'mo_b1', 'mo_s0', 'mo_tr0', 'idx_out_l', 'idx_out_r', 'idx_act_l',
    'idx_act_r',
)

_pmapped = jax.pmap(
    _shard_forward,
    in_axes=(0, 0) + (None,) * len(_WEIGHT_ORDER),
    devices=jax.devices()[:N_CORES],
)

_compiled = False


def kernel(**inputs):
    global _compiled
    x = np.asarray(inputs['x'])
    # Data-parallel over BT: core c -> batch c // 4, token rows
    # [128*(c%4), 128*(c%4)+128). Weights replicated on every core.
    x_shards = np.stack([x[c // (N_CORES // B)] for c in range(N_CORES)])
    rows = np.stack([
        (c % (N_CORES // B)) * TCH + np.arange(TCH, dtype=np.int32)
        for c in range(N_CORES)
    ])
    weights = [np.asarray(inputs[k]) for k in _WEIGHT_ORDER]
    out_shards = _pmapped(x_shards, rows, *weights)  # (8, TCH, D)
    out_shards = np.asarray(jax.device_get(out_shards))
    _compiled = True
    per_batch = N_CORES // B
    out = np.concatenate(
        [
            out_shards[b * per_batch:(b + 1) * per_batch].reshape(T, D)[None]
            for b in range(B)
        ],
        axis=0,
    )
    return out.astype(np.float32)
